# revision 1
# baseline (speedup 1.0000x reference)
"""LBQuantization Trainium2 Bass kernel (nn_LBQuantization_35021163331684).

Math per channel (C = B*c = 96, HW = 512*512 px):
    mn, mx = min(x_ch), max(x_ch)
    t_j = rp_j * (mx - mn) + mn   (rp pre-sorted on host, j = 1..7)
    out = largest v in {mn, t_1..t_7} with v <= pixel

Device decomposition per channel tile [128, 2048]:
  DVE:  1. fused min/max custom op (out stream = running max of -x through
           a stride-0 AP; accum rewired to fold raw x -> per-lane max)
        2. thresholds: pos = rp*rng + mn via custom AFFINE (exact
           mul-then-add; ACT Identity's multiply is not correctly rounded)
        3. select chain, 2-3 exact compare-thresholds per op (the v3
           custom-DVE slot limit):
              r1 = x>=t3?t3 : x>=t2?t2 : x>=t1?t1 : -FLT_MAX   (SEL3N)
              r2 = x>=t5?t5 : x>=t4?t4 : r1                    (SEL2C)
              r3 = x>=t7?t7 : x>=t6?t6 : r2                    (SEL2C, bf16 out)
  GPS:  cross-partition all-reduce of the min/max partials, and the
        mn-patch out = max(r3, mn) (tensor_scalar_max, the only legal
        full-width GPSIMD form: 2-tensor ops do not exist on Pool).
  ACT:  mn = -(-mn), rng = mx + (-mn) (exact negate / bias-add).

The final select link and the patch write bf16: every compare is an exact
fp32 is_ge against the exact threshold, so bucketing matches the
reference exactly; only the output VALUES carry the bf16 rounding
(rel err ~3e-3, far under the 2e-2 gate). bf16 also halves write DMA.

Schedule: channels are software-pipelined one deep -- channel c+1's
DMA + min/max + all-reduce are emitted in the middle of channel c's
select chain, so the DVE never waits on the GPS->ACT->DVE threshold
round-trip (steady-state DVE occupancy ~93%, zero mid-stream gaps).
Channel 0 is scanned in ramped column chunks; the last channel drains
in shrinking column slices whose patches run on the DVE at the 4x bf16
tensor_scalar rate, overlapping the tail out-DMA.

Sharding: fully data-parallel, 12 channels/core x 8 cores, no
collectives. TimelineSim cost-model estimate: ~115.3 us/core
(baseline 119.5).
"""

import sys

if "/opt/trn_rl_repo" not in sys.path:
    sys.path.insert(0, "/opt/trn_rl_repo")

import numpy as np

N_CORES = 8
B, CC, H, W = 32, 3, 512, 512
C_TOTAL = B * CC          # 96
C_PER = C_TOTAL // N_CORES  # 12
P = 128
FD = (H * W) // P         # 2048
R = 8                     # region_num

_CACHE: dict = {}


# --------------------------------------------------------------------------- #
# Custom DVE ops (SEL3N / AFFINE / MINMAX, from the tuned baseline)
# --------------------------------------------------------------------------- #
def _register_ops():
    if "ops" in _CACHE:
        return _CACHE["ops"]
    from concourse import dve_ops
    from concourse.dve_spec import (
        C0,
        C1,
        C3,
        AluOp,
        MaxNeg,
        Spec,
        Src0,
        Src1,
        Zero,
        _spill_c3_to_src1,
        lower,
        scan,
        select,
    )
    from concourse.dve_uop import AluInp, DveOpSpec

    def mk(name, spec):
        if name in dve_ops._SUB_OPCODE_FOR_NAME:
            return next(op for op in dve_ops.OPS if op.name == name)
        row = dve_ops._CUSTOM_DVE_ROW_BASE + len(dve_ops.OPS)
        assert row < 0x20, "custom DVE opcode rows exhausted"
        dve_ops._SUB_OPCODE_FOR_NAME[name] = row
        shas = {}
        for ver in ("v3", "v4"):
            try:
                shas[ver] = DveOpSpec(
                    name=name,
                    opcode=row,
                    uops=lower(spec, ver=ver),
                    rd1_en=dve_ops.has_src1(spec),
                ).sha(ver)
            except ValueError:
                pass
        assert "v3" in shas, f"{name}: v3 lowering failed"
        op = dve_ops.DveOp(name, spec, subdim=False, uops_sha=shas)
        dve_ops.OPS.append(op)
        dve_ops.CUSTOM_DVE_SPECS[name] = spec
        return op

    # r = x>=t_c ? t_c : (x>=t_b ? t_b : (x>=t_a ? t_a : -FLT_MAX))
    # [t_c via C3 -> in1 [P,1]]
    sel3 = mk(
        "LBQ_SEL3N",
        Spec(
            body=_spill_c3_to_src1(
                select(
                    Src0 >= C3,
                    C3,
                    select(Src0 >= C1, C1, select(Src0 >= C0, C0, MaxNeg)),
                )
            ),
            reference=lambda in0, in1, c0, c1, c2: np.where(
                in0 >= in1, in1,
                np.where(
                    in0 >= c1, c1,
                    np.where(in0 >= c0, c0, np.float32(-3.4028235e38)),
                ),
            ).astype(np.float32),
        ),
    )
    # r = x>=t_b ? t_b : (x>=t_a ? t_a : carry)   [carry via Src1 [P,N]]
    sel2c = mk(
        "LBQ_SEL2C",
        Spec(
            body=select(Src0 >= C1, C1, select(Src0 >= C0, C0, Src1)),
            reference=lambda in0, in1, c0, c1, c2: np.where(
                in0 >= c1, c1, np.where(in0 >= c0, c0, in1)
            ).astype(np.float32),
        ),
    )
    # pos = rp*rng + mn (exact mul-then-add on the DVE datapath)
    affine = mk(
        "LBQ_AFFINE",
        Spec(
            body=Src0 * C0 + C1,
            reference=lambda in0, in1, c0, c1, c2: (
                in0.astype(np.float32) * c0 + c1
            ).astype(np.float32),
        ),
    )

    # Single-pass dual min/max (see baseline docstring): out stream is the
    # running max of -x drained through a stride-0 AP (only -min lands);
    # the accum stage is rewired post-lowering to fold raw Src0 -> max(x).
    def _minmax_ref(in0, in1, c0, c1, c2):
        x = in0.astype(np.float32)
        negmins = np.maximum.accumulate(np.maximum(-x, np.float32(c0)), axis=-1)
        mx = x.reshape(x.shape[0], -1).max(axis=-1, keepdims=True)
        return negmins, np.maximum(mx, np.float32(-3.4028235e38))

    mm_name = "LBQ_MINMAX"
    if mm_name not in dve_ops._SUB_OPCODE_FOR_NAME:
        mm_spec = Spec(
            body=scan(AluOp.MAX, Zero - Src0, init=C0),
            accum=AluOp.MAX,
            reference=_minmax_ref,
        )
        row = dve_ops._CUSTOM_DVE_ROW_BASE + len(dve_ops.OPS)
        assert row < 0x20
        dve_ops._SUB_OPCODE_FOR_NAME[mm_name] = row
        uops = lower(mm_spec, ver="v3")
        steady = uops[-1]
        acc_st = None
        src0_lane = None
        for st, dp in enumerate(steady.datapath_config):
            if int(dp.alu_out_a_enable):
                assert dp.op == AluOp.MAX and dp.alu_src1 == AluInp.PREV_ALU_OUT
                acc_st = st
                break
        for lane_idx in range(1, 7):
            if int(steady.inp_enable[lane_idx]) and steady.inp[lane_idx].name == "SRC_0":
                src0_lane = lane_idx - 1
                break
        assert acc_st is not None and src0_lane is not None, (acc_st, src0_lane)
        steady.datapath_config[acc_st].alu_src1 = AluInp(
            int(AluInp.PREV_DELAY_0) + src0_lane
        )
        compiled = DveOpSpec(name=mm_name, opcode=row, uops=uops, rd1_en=False)
        minmax = dve_ops.DveOp(
            mm_name,
            mm_spec,
            subdim=False,
            uops_sha={"v3": compiled.sha("v3")},
        )
        dve_ops._COMPILE_CACHE[(mm_name, "v3")] = compiled
        dve_ops.OPS.append(minmax)
        dve_ops.CUSTOM_DVE_SPECS[mm_name] = mm_spec
    else:
        minmax = next(op for op in dve_ops.OPS if op.name == mm_name)

    _CACHE["ops"] = (sel3, sel2c, affine, minmax)
    return _CACHE["ops"]


# --------------------------------------------------------------------------- #
# Bass module (SPMD: same program on all 8 cores, different data)
# --------------------------------------------------------------------------- #
def _build_module():
    if "nc" in _CACHE:
        return _CACHE["nc"]
    import concourse.bacc as bacc
    import concourse.bass as bass
    import concourse.bass_isa as bass_isa
    import concourse.tile as tile
    from concourse import mybir

    SEL3, SEL2C, AFFINE, MINMAX = _register_ops()
    f32 = mybir.dt.float32
    bf16 = mybir.dt.bfloat16
    FLT_MAX = 3.4028234663852886e38

    nc = bacc.Bacc("TRN2", target_bir_lowering=False, name="lbq4")
    x_d = nc.dram_tensor("x", [C_PER, P, FD], f32, kind="ExternalInput")
    rp_d = nc.dram_tensor("rp", [C_PER, R - 1], f32, kind="ExternalInput")
    y_d = nc.dram_tensor("y", [C_PER, P, FD], bf16, kind="ExternalOutput")

    with tile.TileContext(nc) as tc:
        with (
            tc.tile_pool(name="xp", bufs=3) as xp,
            tc.tile_pool(name="wp", bufs=3) as wp,
            tc.tile_pool(name="sp", bufs=1) as sp,
            tc.tile_pool(name="op", bufs=3) as op_,
        ):
            # rp [12,7] DRAM -> one SBUF row -> gpsimd broadcast to [128, 84]
            # (emitted after channel 0's scan DMAs so it stays off the ramp
            # critical path)
            rp_b = sp.tile([P, C_PER, R - 1], f32, tag="rp_b")

            def emit_rp():
                rp_row = sp.tile([1, C_PER * (R - 1)], f32, tag="rp_row")
                rp_ap = rp_d[:, :]
                nc.sync.dma_start(
                    out=rp_row,
                    in_=bass.AP(
                        tensor=rp_ap.tensor,
                        offset=rp_ap.offset,
                        ap=[[0, 1], [1, C_PER * (R - 1)]],
                    ),
                )
                nc.gpsimd.partition_broadcast(
                    rp_b.rearrange("p c r -> p (c r)"), rp_row, channels=P
                )

            def minmax_sink(dst_negmin, dst_max, src, fd):
                sink = bass.AP(
                    tensor=dst_negmin.tensor,
                    offset=dst_negmin.offset,
                    ap=[list(dst_negmin.ap[0]), [0, fd]],
                )
                nc.vector._custom_dve(
                    MINMAX, out=sink, in0=src,
                    s0=-FLT_MAX, accum_out=dst_max,
                )

            def load_and_scan(c):
                """DMA channel c in, fused min/max scan, cross-partition
                all-reduce. Emitted one channel ahead of the select chain."""
                xt = xp.tile([P, FD], f32, tag="x")
                pm = sp.tile([P, 2], f32, tag=f"pm{c}")
                if c == 0:
                    # channel 0 gates the ramp: load + scan in column
                    # chunks so the first compute starts early.
                    bounds = [0, 448, 1024, 1600, FD]
                    n_ck = len(bounds) - 1
                    pm8 = sp.tile([P, 2, n_ck], f32, tag="pm_ck")
                    for i in range(n_ck):
                        sl = slice(bounds[i], bounds[i + 1])
                        nc.sync.dma_start(out=xt[:, sl], in_=x_d[c][:, sl])
                        minmax_sink(
                            pm8[:, 0, i : i + 1], pm8[:, 1, i : i + 1],
                            xt[:, sl], bounds[i + 1] - bounds[i],
                        )
                    nc.vector.tensor_reduce(
                        out=pm[:, 0:1], in_=pm8[:, 0, :],
                        axis=mybir.AxisListType.X, op=mybir.AluOpType.max,
                    )
                    nc.vector.tensor_reduce(
                        out=pm[:, 1:2], in_=pm8[:, 1, :],
                        axis=mybir.AxisListType.X, op=mybir.AluOpType.max,
                    )
                else:
                    nc.sync.dma_start(out=xt, in_=x_d[c])
                    minmax_sink(pm[:, 0:1], pm[:, 1:2], xt, FD)

                # ar[:,0] = -mn, ar[:,1] = mx (broadcast to all partitions)
                ar = sp.tile([P, 2], f32, tag=f"ar{c}")
                nc.gpsimd.partition_all_reduce(
                    ar, pm, P, bass_isa.ReduceOp.max
                )
                return xt, ar

            def process(c, xt, thr, mid_emit=None, mid_emit2=None):
                """Threshold chain + selects + patch + out-DMA for channel
                c (runs while channel c+1 is being scanned)."""
                # chained selects (the only HW-legal 2-thresholds-per-op
                # form); the last link converts to bf16 on write, and the
                # mn-patch runs on the otherwise idle GPSIMD engine.
                c1t = wp.tile([P, FD], f32, tag="c1")
                c2t = wp.tile([P, FD], f32, tag="c2")
                rbt = wp.tile([P, FD], bf16, tag="rb")
                ot = op_.tile([P, FD], bf16, tag="out")

                first = [True]

                def body(lo, hi, dve_patch=False):
                    """Select chain + patch + out-DMA for columns [lo, hi)."""
                    sl = slice(lo, hi)
                    nc.vector._custom_dve(
                        SEL3, out=c1t[:, sl], in0=xt[:, sl], in1=thr[:, 3:4],
                        s0=thr[:, 1:2], s1=thr[:, 2:3],
                    )
                    if first[0] and mid_emit is not None:
                        # emit the next channel's scan here so its min/max
                        # runs while this channel's carries resolve
                        mid_emit()
                    nc.vector._custom_dve(
                        SEL2C, out=c2t[:, sl], in0=xt[:, sl], in1=c1t[:, sl],
                        s0=thr[:, 4:5], s1=thr[:, 5:6],
                    )
                    if first[0] and mid_emit2 is not None:
                        # emit the next channel's threshold chain here so the
                        # AFFINE->select pipeline drain hides under the last
                        # carry link instead of stalling the DVE
                        mid_emit2()
                    first[0] = False
                    nc.vector._custom_dve(
                        SEL2C, out=rbt[:, sl], in0=xt[:, sl], in1=c2t[:, sl],
                        s0=thr[:, 6:7], s1=thr[:, 7:8],
                    )
                    # patch: out = max(r, mn); bf16 in/out.  DVE runs this
                    # at the 4x tensor_scalar rate for tail slices.
                    eng = nc.vector if dve_patch else nc.gpsimd
                    eng.tensor_scalar_max(ot[:, sl], rbt[:, sl], thr[:, 0:1])
                    nc.sync.dma_start(out=y_d[c][:, sl], in_=ot[:, sl])

                if c == C_PER - 1:
                    # the last channel gates the kernel tail: drain in
                    # shrinking slices; Pool patches the first (it is free
                    # by then), DVE the rest at the 4x bf16 rate
                    t11 = [0, 1024, 1664, FD]
                    for i in range(len(t11) - 1):
                        body(t11[i], t11[i + 1], dve_patch=(i > 0))
                elif c == C_PER - 2:
                    # penultimate channel: sliced so its patch + DMA
                    # drain before the final channel's
                    t10 = [0, FD]
                    for i in range(len(t10) - 1):
                        body(t10[i], t10[i + 1])
                else:
                    body(0, FD)

            def thr_prep(c, ar):
                """mn / rng / thresholds for channel c (ACT + tiny DVE op).
                Emitted inside channel c-1's select chain."""
                thr = sp.tile([P, R], f32, tag=f"thr{c}")
                rng = sp.tile([P, 1], f32, tag=f"rng{c}")
                nc.scalar.activation(
                    out=thr[:, 0:1], in_=ar[:, 0:1],
                    func=mybir.ActivationFunctionType.Copy, scale=-1.0,
                )
                nc.scalar.activation(
                    out=rng, in_=ar[:, 1:2],
                    func=mybir.ActivationFunctionType.Identity,
                    bias=ar[:, 0:1], scale=1.0,
                )
                # pos = rp*rng + mn on DVE (exact; ACT Identity is not
                # correctly rounded for the multiply)
                nc.vector._custom_dve(
                    AFFINE, out=thr[:, 1:R], in0=rp_b[:, c, :],
                    s0=rng[:, 0:1], s1=thr[:, 0:1],
                )
                return thr

            xt0, ar0 = load_and_scan(0)
            emit_rp()
            prev = (xt0, thr_prep(0, ar0))
            for c in range(1, C_PER):
                holder = {}

                def hook1(c=c, holder=holder):
                    holder["ls"] = load_and_scan(c)

                def hook2(c=c, holder=holder):
                    holder["thr"] = thr_prep(c, holder["ls"][1])

                process(c - 1, prev[0], prev[1], mid_emit=hook1,
                        mid_emit2=hook2)
                prev = (holder["ls"][0], holder["thr"])
            process(C_PER - 1, prev[0], prev[1])

    nc.compile()
    _CACHE["nc"] = nc
    return nc


# --------------------------------------------------------------------------- #
# Host entry point
# --------------------------------------------------------------------------- #
def kernel(x, region_percentiles, _trace=False):
    x = np.asarray(x)
    in_dtype = x.dtype
    xs = np.ascontiguousarray(x, dtype=np.float32).reshape(
        N_CORES, C_PER, P, FD
    )
    rp = np.sort(
        np.ascontiguousarray(region_percentiles, dtype=np.float32), axis=1
    ).reshape(N_CORES, C_PER, R - 1)

    nc = _build_module()
    from concourse.bass_utils import run_bass_kernel_spmd

    in_maps = [{"x": xs[i], "rp": np.ascontiguousarray(rp[i])} for i in range(N_CORES)]
    res = run_bass_kernel_spmd(
        nc, in_maps, core_ids=list(range(N_CORES)), trace=_trace
    )
    _CACHE["last_result"] = res
    y = np.stack([np.asarray(res.results[i]["y"]) for i in range(N_CORES)])
    return y.reshape(B, CC, H, W).astype(in_dtype)

